# revision 10
# baseline (speedup 1.0000x reference)
"""InstanceConv (mask-gated 3x3 conv) Trainium2 kernel.

Full inputs: inp [8,64,128,128] f32, mask [8,128,128] f32,
conv [64,64,3,3] f32, bias [64] f32.
Returns (out [8,64,128,128] f32, mask_out [8,1,128,128] f32).

Sharding: data-parallel over batch, one image per NeuronCore (8 cores),
conv weights/bias replicated. mask_out is the center tap of the unfolded
mask which, at stride 1 / pad 1, is the input mask itself (reshaped on
host; no device work needed).

Per-core algorithm (positions p = (h, w), C=64, OC=64, K=3x3):
  eq_t[p]  = (mask[p + off_t] == mask[p])        (pad -1 -> 0 at borders)
  norm[p]  = 9 / sum_t eq_t[p]
  out[o,p] = sum_t sum_c w[o,c,t] * inp[c, p+off_t] * eq_t[p] * norm[p]
           + bias[o]
The norm factor is folded into per-tap gate maps geq_t = 9*norm*eq_t
(center tap: geq = 9*norm). Taps are packed in pairs so each matmul
contracts K=128 = 2 taps x 64 channels; the gate maps are broadcast
across the 64 channel partitions with K=2 ones-matmuls on the tensor
engine (output in PSUM), and the masked moving operand is built with
vector-engine multiplies.
"""

import numpy as np
import sys

if "/opt/trn_rl_repo" not in sys.path:
    sys.path.insert(0, "/opt/trn_rl_repo")

import concourse.bass as bass
import concourse.bacc as bacc
import concourse.mybir as mybir
from concourse.mybir import AluOpType, ActivationFunctionType
from concourse.tile import TileContext

F32 = mybir.dt.float32
BF16 = mybir.dt.bfloat16

C = 64          # input channels
OC = 64         # output channels
H = 128
W = 128
P = H * W       # 16384 positions
WP = W + 2      # padded width 130
HP = H + 2
NPAD = WP * HP  # padded flat size 16900
CHUNK = 512     # output positions per chunk (4 image rows)
NCHUNK = P // CHUNK
ROWS_PER_CHUNK = CHUNK // W  # 4

# tap index t = kh*3 + kw ; padded flat offset of tap = kh*WP + kw
# groups: 4 pairs (shift delta 1 or WP) + 1 single (tap 8 = (2,2))
# pair (a, b) requires off_b - off_a == delta of the input replica used.
GROUPS = [
    ((0, 0), (0, 1), 1),    # g0: taps (0,0),(0,1)   delta 1   -> inp2
    ((1, 0), (1, 1), 1),    # g1: taps (1,0),(1,1)   delta 1   -> inp2
    ((2, 0), (2, 1), 1),    # g2: taps (2,0),(2,1)   delta 1   -> inp2
    ((0, 2), (1, 2), WP),   # g3: taps (0,2),(1,2)   delta 130 -> inp3
    ((2, 2), None, None),   # g4: tap (2,2) single
]
# partition bases for the 4 pair gate-row slots (PE row-group tiling)
PAIR_BASES = [0, 32, 64, 96]


def tap_off(kh, kw):
    return kh * WP + kw


def _build(nc):
    inp_d = nc.dram_tensor("inp", [C, P], F32, kind="ExternalInput")
    mask_d = nc.dram_tensor("mask", [H, W], F32, kind="ExternalInput")
    wpack_d = nc.dram_tensor("wpack", [128, 5 * OC], F32, kind="ExternalInput")
    bias_d = nc.dram_tensor("bias", [OC, 1], F32, kind="ExternalInput")
    out_d = nc.dram_tensor("out", [OC, P], F32, kind="ExternalOutput")

    with TileContext(nc) as tc:
        with (
            tc.tile_pool(name="const", bufs=1) as cpool,
            tc.tile_pool(name="work", bufs=3) as wpool,
            tc.tile_pool(name="psum_eq", bufs=1, space="PSUM") as eqpool,
            tc.tile_pool(name="psum_acc", bufs=2, space="PSUM") as accpool,
        ):
            # ---------- constants / stage 0 ----------
            w_sb = cpool.tile([128, 5 * OC], BF16, tag="w_sb")
            nc.gpsimd.dma_start(out=w_sb[:, :], in_=wpack_d[:, :])  # casting DMA

            bias_sb = cpool.tile([OC, 1], F32, tag="bias_sb")
            nc.sync.dma_start(out=bias_sb[:, :], in_=bias_d[:, :])

            # block-diagonal selector for the gate-broadcast matmuls:
            # row B   = [1]*64 + [0]*64  -> tap_a gate lands on partitions 0:64
            # row B+1 = [0]*64 + [1]*64  -> tap_b gate lands on partitions 64:128
            sel_sb = cpool.tile([128, 128], BF16, tag="sel_sb")
            ones_row = cpool.tile([1, 64], BF16, tag="ones_row")
            nc.vector.memset(sel_sb[:, :], 0.0)
            nc.vector.memset(ones_row[:, :], 1.0)
            for B in PAIR_BASES:
                nc.sync.dma_start(out=sel_sb[B : B + 1, 0:64], in_=ones_row[:, :])
                nc.sync.dma_start(out=sel_sb[B + 1 : B + 2, 64:128], in_=ones_row[:, :])

            # padded input, bf16, two stacked replicas:
            # inp2: partitions 0:64 raw padded, 64:128 shifted by +1
            # inp3: partitions 0:64 raw padded, 64:128 shifted by +WP
            inp2 = cpool.tile([128, NPAD], BF16, tag="inp2")
            inp3 = cpool.tile([128, NPAD], BF16, tag="inp3")
            nc.gpsimd.memset(inp2[:, :], 0.0)
            nc.gpsimd.memset(inp3[:, :], 0.0)
            # cast-DMA the image into the padded interior of inp2[0:64]
            inp2_int = inp2[0:C].rearrange("c (h w) -> c h w", h=HP)[
                :, 1 : H + 1, 1 : W + 1
            ]
            nc.gpsimd.dma_start(
                out=inp2_int, in_=inp_d[:, :].rearrange("c (h w) -> c h w", h=H)
            )
            # on-chip shifted/raw copies (bf16)
            nc.vector.tensor_copy(out=inp2[64:128, 0:16898], in_=inp2[0:C, 1:16899])
            nc.vector.tensor_copy(out=inp3[0:C, :], in_=inp2[0:C, :])
            nc.vector.tensor_copy(
                out=inp3[64:128, 0 : NPAD - WP], in_=inp2[0:C, WP:NPAD]
            )

            # ---------- stage 1: gate maps in [h, w] layout ----------
            # width-padded mask copies, fill -1 so borders compare unequal
            mk0 = cpool.tile([128, WP], F32, tag="mk0")   # mask[h, w-1+c]
            mkm = cpool.tile([128, WP], F32, tag="mkm")   # mask[h-1, ...]
            mkp = cpool.tile([128, WP], F32, tag="mkp")   # mask[h+1, ...]
            for t in (mk0, mkm, mkp):
                nc.vector.memset(t[:, :], -1.0)
            nc.sync.dma_start(out=mk0[:, 1 : W + 1], in_=mask_d[:, :])
            nc.sync.dma_start(out=mkm[1:128, 1 : W + 1], in_=mask_d[0 : H - 1, :])
            nc.sync.dma_start(out=mkp[0:127, 1 : W + 1], in_=mask_d[1:H, :])
            mk_by_kh = {0: mkm, 1: mk0, 2: mkp}

            # eq maps for the 8 non-center taps, then count and norm
            eqs = {}
            for kh in range(3):
                for kw in range(3):
                    if (kh, kw) == (1, 1):
                        continue
                    eq = cpool.tile([128, W], F32, tag=f"eq_{kh}{kw}")
                    nc.vector.tensor_tensor(
                        out=eq[:, :],
                        in0=mk_by_kh[kh][:, kw : kw + W],
                        in1=mk0[:, 1 : W + 1],
                        op=AluOpType.is_equal,
                    )
                    eqs[(kh, kw)] = eq

            count = cpool.tile([128, W], F32, tag="count")
            eql = list(eqs.values())
            nc.vector.tensor_add(out=count[:, :], in0=eql[0][:, :], in1=eql[1][:, :])
            for e in eql[2:]:
                nc.vector.tensor_add(out=count[:, :], in0=count[:, :], in1=e[:, :])
            # + center tap
            nc.vector.tensor_scalar_add(count[:, :], count[:, :], 1.0)
            norm = cpool.tile([128, W], F32, tag="norm")
            nc.vector.reciprocal(out=norm[:, :], in_=count[:, :])

            # gate maps geq = (9 * norm) * eq  (center: 9 * norm), bf16
            geqs = {}
            for (kh, kw), eq in eqs.items():
                g = cpool.tile([128, W], BF16, tag=f"geq_{kh}{kw}")
                nc.vector.scalar_tensor_tensor(
                    out=g[:, :],
                    in0=norm[:, :],
                    scalar=9.0,
                    in1=eq[:, :],
                    op0=AluOpType.mult,
                    op1=AluOpType.mult,
                )
                geqs[(kh, kw)] = g
            gc = cpool.tile([128, W], BF16, tag="geq_11")
            nc.vector.tensor_single_scalar(
                out=gc[:, :], in_=norm[:, :], scalar=9.0, op=AluOpType.mult
            )
            geqs[(1, 1)] = gc

            # ---------- stage 2: flatten gate maps into broadcast rows ----------
            # pair rows at partitions (B, B+1) for B in PAIR_BASES; single at
            # rows2[0]. SBUF->SBUF DMA flattens [128, W] -> [1, P].
            rows = cpool.tile([128, P], BF16, tag="rows")
            rows2 = cpool.tile([1, P], BF16, tag="rows2")
            for g, (ta, tb, _delta) in enumerate(GROUPS):
                if tb is None:
                    nc.sync.dma_start(out=rows2[0:1, :], in_=geqs[ta][:, :])
                else:
                    B = PAIR_BASES[g]
                    nc.sync.dma_start(out=rows[B : B + 1, :], in_=geqs[ta][:, :])
                    nc.sync.dma_start(out=rows[B + 1 : B + 2, :], in_=geqs[tb][:, :])

            # ---------- stage 3: chunk loop ----------
            for ci in range(NCHUNK):
                h0 = ci * ROWS_PER_CHUNK
                c0 = ci * CHUNK

                # broadcast gate rows across partitions via ones-matmuls
                psum_eqs = []
                for g, (ta, tb, _delta) in enumerate(GROUPS[:4]):
                    B = PAIR_BASES[g]
                    pe = eqpool.tile([128, CHUNK], F32, tag=f"psum_eq{g}")
                    nc.tensor.matmul(
                        out=pe[:, :],
                        lhsT=sel_sb[B : B + 2, 0:128],
                        rhs=rows[B : B + 2, c0 : c0 + CHUNK],
                        start=True,
                        stop=True,
                        tile_position=(B, 0),
                    )
                    psum_eqs.append(pe)
                pe4 = eqpool.tile([64, CHUNK], F32, tag="psum_eq4")
                nc.tensor.matmul(
                    out=pe4[:, :],
                    lhsT=sel_sb[0:1, 0:64],
                    rhs=rows2[0:1, c0 : c0 + CHUNK],
                    start=True,
                    stop=True,
                )

                # masked moving operands (bf16) and conv matmuls
                acc = accpool.tile([OC, CHUNK], F32, tag="acc")
                for g, (ta, tb, delta) in enumerate(GROUPS):
                    kh, kw = ta
                    src = inp3 if delta == WP else inp2
                    kpart = 128 if tb is not None else 64
                    in0 = src[0:kpart].rearrange("c (h w) -> c h w", h=HP)
                    in0 = in0[:, h0 + kh : h0 + kh + ROWS_PER_CHUNK, kw : kw + W]
                    peq = psum_eqs[g] if tb is not None else pe4
                    in1 = peq[0:kpart].rearrange("c (r w) -> c r w", r=ROWS_PER_CHUNK)
                    m = wpool.tile([kpart, CHUNK], BF16, tag=f"masked{g}")
                    mv = m[:, :].rearrange("c (r w) -> c r w", r=ROWS_PER_CHUNK)
                    nc.vector.tensor_tensor(
                        out=mv, in0=in0, in1=in1, op=AluOpType.mult
                    )
                    nc.tensor.matmul(
                        out=acc[:, :],
                        lhsT=w_sb[0:kpart, g * OC : (g + 1) * OC],
                        rhs=m[:, :],
                        start=(g == 0),
                        stop=(g == len(GROUPS) - 1),
                    )

                # bias + copy out
                ob = wpool.tile([OC, CHUNK], F32, tag="out_sb")
                nc.scalar.activation(
                    out=ob[:, :],
                    in_=acc[:, :],
                    func=ActivationFunctionType.Identity,
                    bias=bias_sb[:, :],
                    scale=1.0,
                )
                nc.sync.dma_start(out=out_d[:, c0 : c0 + CHUNK], in_=ob[:, :])

    nc.compile()
    return nc


_NC_CACHE = None


def get_nc():
    global _NC_CACHE
    if _NC_CACHE is None:
        nc = bacc.Bacc("TRN2", target_bir_lowering=False)
        _NC_CACHE = _build(nc)
    return _NC_CACHE


def make_wpack(conv):
    """[128, 5*OC] f32: group g cols, rows 0:64 = tap_a weights (w[o,c,a]
    transposed to [c, o]), rows 64:128 = tap_b weights (zero for single)."""
    wpack = np.zeros((128, 5 * OC), dtype=np.float32)
    for g, (ta, tb, _d) in enumerate(GROUPS):
        wpack[0:C, g * OC : (g + 1) * OC] = conv[:, :, ta[0], ta[1]].T
        if tb is not None:
            wpack[64:128, g * OC : (g + 1) * OC] = conv[:, :, tb[0], tb[1]].T
    return wpack


def kernel(inp, mask, conv, bias):
    from concourse.bass_utils import run_bass_kernel_spmd

    inp = np.ascontiguousarray(inp, dtype=np.float32)
    mask = np.ascontiguousarray(mask, dtype=np.float32)
    conv = np.ascontiguousarray(conv, dtype=np.float32)
    bias = np.ascontiguousarray(bias, dtype=np.float32)
    B = inp.shape[0]

    nc = get_nc()
    wpack = make_wpack(conv)
    in_maps = [
        {
            "inp": inp[b].reshape(C, P),
            "mask": mask[b],
            "wpack": wpack,
            "bias": bias.reshape(OC, 1),
        }
        for b in range(B)
    ]
    res = run_bass_kernel_spmd(nc, in_maps, core_ids=list(range(B)))
    out = np.stack([res.results[b]["out"].reshape(OC, H, W) for b in range(B)])
    mask_out = mask.reshape(B, 1, H, W).copy()
    return out, mask_out


if __name__ == "__main__":
    # smoke build
    get_nc()
    print("build ok")


# revision 14
# speedup vs baseline: 181.8329x; 181.8329x over previous
"""InstanceConv (mask-gated 3x3 conv) Trainium2 kernel.

Full inputs: inp [8,64,128,128] f32, mask [8,128,128] f32,
conv [64,64,3,3] f32, bias [64] f32.
Returns (out [8,64,128,128] f32, mask_out [8,1,128,128] f32).

Sharding: data-parallel over batch, one image per NeuronCore (8 cores),
conv weights/bias replicated. mask_out is the center tap of the unfolded
mask which, at stride 1 / pad 1, is the input mask itself (reshaped on
host; no device work needed).

Per-core algorithm (positions p = (h, w), C=64, OC=64, K=3x3):
  eq_t[p]  = (mask[p + off_t] == mask[p])        (pad -1 -> 0 at borders)
  norm[p]  = 9 / sum_t eq_t[p]
  out[o,p] = sum_t sum_c w[o,c,t] * inp[c, p+off_t] * eq_t[p] * norm[p]
           + bias[o]
The norm factor is folded into per-tap gate maps geq_t = 9*norm*eq_t
(center tap: geq = 9*norm). Taps are packed in pairs so each matmul
contracts K=128 = 2 taps x 64 channels; the gate maps are broadcast
across the 64 channel partitions with K=2 ones-matmuls on the tensor
engine (output in PSUM), and the masked moving operand is built with
vector-engine multiplies.
"""

import numpy as np
import sys

if "/opt/trn_rl_repo" not in sys.path:
    sys.path.insert(0, "/opt/trn_rl_repo")

import concourse.bass as bass
import concourse.bacc as bacc
import concourse.mybir as mybir
from concourse.mybir import AluOpType, ActivationFunctionType
from concourse.tile import TileContext

F32 = mybir.dt.float32
BF16 = mybir.dt.bfloat16

C = 64          # input channels
OC = 64         # output channels
H = 128
W = 128
P = H * W       # 16384 positions
WP = W + 2      # padded width 130
HP = H + 2
NPAD = WP * HP  # padded flat size 16900
CHUNK = 512     # output positions per chunk (4 image rows)
NCHUNK = P // CHUNK
ROWS_PER_CHUNK = CHUNK // W  # 4

# tap index t = kh*3 + kw ; padded flat offset of tap = kh*WP + kw
# groups: 4 pairs (shift delta 1 or WP) + 1 single (tap 8 = (2,2))
# pair (a, b) requires off_b - off_a == delta of the input replica used.
GROUPS = [
    ((0, 0), (0, 1), 1),    # g0: taps (0,0),(0,1)   delta 1   -> inp2
    ((1, 0), (1, 1), 1),    # g1: taps (1,0),(1,1)   delta 1   -> inp2
    ((2, 0), (2, 1), 1),    # g2: taps (2,0),(2,1)   delta 1   -> inp2
    ((0, 2), (1, 2), WP),   # g3: taps (0,2),(1,2)   delta 130 -> inp3
    ((2, 2), None, None),   # g4: tap (2,2) single
]
# partition bases for the 4 pair gate-row slots (PE row-group tiling)
PAIR_BASES = [0, 32, 64, 96]


def tap_off(kh, kw):
    return kh * WP + kw


def _build(nc, repeat=1):
    inp_d = nc.dram_tensor("inp", [C, P], F32, kind="ExternalInput")
    mask_d = nc.dram_tensor("mask", [H, W], F32, kind="ExternalInput")
    wpack_d = nc.dram_tensor("wpack", [128, 5 * OC], F32, kind="ExternalInput")
    bias_d = nc.dram_tensor("bias", [OC, 1], F32, kind="ExternalInput")
    out_d = nc.dram_tensor("out", [OC, P], F32, kind="ExternalOutput")

    with TileContext(nc) as tc:
        with (
            tc.tile_pool(name="const", bufs=1) as cpool,
            tc.tile_pool(name="work", bufs=3) as wpool,
            tc.tile_pool(name="psum_eq", bufs=1, space="PSUM") as eqpool,
            tc.tile_pool(name="psum_acc", bufs=2, space="PSUM") as accpool,
        ):
          for _rep in range(repeat):
            # ---------- constants / stage 0 ----------
            w_sb = cpool.tile([128, 5 * OC], BF16, tag="w_sb")
            nc.gpsimd.dma_start(out=w_sb[:, :], in_=wpack_d[:, :])  # casting DMA

            bias_sb = cpool.tile([OC, 1], F32, tag="bias_sb")
            nc.sync.dma_start(out=bias_sb[:, :], in_=bias_d[:, :])

            # block-diagonal selector for the gate-broadcast matmuls:
            # row B   = [1]*64 + [0]*64  -> tap_a gate lands on partitions 0:64
            # row B+1 = [0]*64 + [1]*64  -> tap_b gate lands on partitions 64:128
            sel_sb = cpool.tile([128, 128], BF16, tag="sel_sb")
            ones_row = cpool.tile([1, 64], BF16, tag="ones_row")
            nc.vector.memset(sel_sb[:, :], 0.0)
            nc.vector.memset(ones_row[:, :], 1.0)
            for B in PAIR_BASES:
                nc.sync.dma_start(out=sel_sb[B : B + 1, 0:64], in_=ones_row[:, :])
                nc.sync.dma_start(out=sel_sb[B + 1 : B + 2, 64:128], in_=ones_row[:, :])

            # padded input, bf16, two stacked replicas:
            # inp2: partitions 0:64 raw padded, 64:128 shifted by +1
            # inp3: partitions 0:64 raw padded, 64:128 shifted by +WP
            inp2 = cpool.tile([128, NPAD], BF16, tag="inp2")
            inp3 = cpool.tile([128, NPAD], BF16, tag="inp3")
            nc.gpsimd.memset(inp2[:, :], 0.0)
            nc.gpsimd.memset(inp3[:, :], 0.0)
            # cast-DMA the image into the padded interior of inp2[0:64]
            inp2_int = inp2[0:C].rearrange("c (h w) -> c h w", h=HP)[
                :, 1 : H + 1, 1 : W + 1
            ]
            nc.gpsimd.dma_start(
                out=inp2_int, in_=inp_d[:, :].rearrange("c (h w) -> c h w", h=H)
            )
            # on-chip shifted/raw copies (bf16)
            nc.vector.tensor_copy(out=inp2[64:128, 0:16898], in_=inp2[0:C, 1:16899])
            nc.vector.tensor_copy(out=inp3[0:C, :], in_=inp2[0:C, :])
            nc.vector.tensor_copy(
                out=inp3[64:128, 0 : NPAD - WP], in_=inp2[0:C, WP:NPAD]
            )

            # ---------- stage 1: gate maps in [h, w] layout ----------
            # width-padded mask copies, fill -1 so borders compare unequal
            mk0 = cpool.tile([128, WP], F32, tag="mk0")   # mask[h, w-1+c]
            mkm = cpool.tile([128, WP], F32, tag="mkm")   # mask[h-1, ...]
            mkp = cpool.tile([128, WP], F32, tag="mkp")   # mask[h+1, ...]
            for t in (mk0, mkm, mkp):
                nc.vector.memset(t[:, :], -1.0)
            nc.sync.dma_start(out=mk0[:, 1 : W + 1], in_=mask_d[:, :])
            nc.sync.dma_start(out=mkm[1:128, 1 : W + 1], in_=mask_d[0 : H - 1, :])
            nc.sync.dma_start(out=mkp[0:127, 1 : W + 1], in_=mask_d[1:H, :])
            mk_by_kh = {0: mkm, 1: mk0, 2: mkp}

            # eq maps for the 8 non-center taps, then count and norm
            eqs = {}
            for kh in range(3):
                for kw in range(3):
                    if (kh, kw) == (1, 1):
                        continue
                    eq = cpool.tile([128, W], F32, tag=f"eq_{kh}{kw}")
                    nc.vector.tensor_tensor(
                        out=eq[:, :],
                        in0=mk_by_kh[kh][:, kw : kw + W],
                        in1=mk0[:, 1 : W + 1],
                        op=AluOpType.is_equal,
                    )
                    eqs[(kh, kw)] = eq

            count = cpool.tile([128, W], F32, tag="count")
            eql = list(eqs.values())
            nc.vector.tensor_add(out=count[:, :], in0=eql[0][:, :], in1=eql[1][:, :])
            for e in eql[2:]:
                nc.vector.tensor_add(out=count[:, :], in0=count[:, :], in1=e[:, :])
            # + center tap
            nc.vector.tensor_scalar_add(count[:, :], count[:, :], 1.0)
            norm = cpool.tile([128, W], F32, tag="norm")
            nc.vector.reciprocal(out=norm[:, :], in_=count[:, :])

            # gate maps geq = (9 * norm) * eq  (center: 9 * norm), bf16
            geqs = {}
            for (kh, kw), eq in eqs.items():
                g = cpool.tile([128, W], BF16, tag=f"geq_{kh}{kw}")
                nc.vector.scalar_tensor_tensor(
                    out=g[:, :],
                    in0=norm[:, :],
                    scalar=9.0,
                    in1=eq[:, :],
                    op0=AluOpType.mult,
                    op1=AluOpType.mult,
                )
                geqs[(kh, kw)] = g
            gc = cpool.tile([128, W], BF16, tag="geq_11")
            nc.vector.tensor_single_scalar(
                out=gc[:, :], in_=norm[:, :], scalar=9.0, op=AluOpType.mult
            )
            geqs[(1, 1)] = gc

            # ---------- stage 2: flatten gate maps into broadcast rows ----------
            # pair rows at partitions (B, B+1) for B in PAIR_BASES; single at
            # rows2[0]. SBUF->SBUF DMA flattens [128, W] -> [1, P].
            rows = cpool.tile([128, P], BF16, tag="rows")
            rows2 = cpool.tile([1, P], BF16, tag="rows2")
            for g, (ta, tb, _delta) in enumerate(GROUPS):
                if tb is None:
                    nc.sync.dma_start(out=rows2[0:1, :], in_=geqs[ta][:, :])
                else:
                    B = PAIR_BASES[g]
                    nc.sync.dma_start(out=rows[B : B + 1, :], in_=geqs[ta][:, :])
                    nc.sync.dma_start(out=rows[B + 1 : B + 2, :], in_=geqs[tb][:, :])

            # ---------- stage 3: chunk loop ----------
            for ci in range(NCHUNK):
                h0 = ci * ROWS_PER_CHUNK
                c0 = ci * CHUNK

                # broadcast gate rows across partitions via ones-matmuls
                psum_eqs = []
                for g, (ta, tb, _delta) in enumerate(GROUPS[:4]):
                    B = PAIR_BASES[g]
                    pe = eqpool.tile([128, CHUNK], F32, tag=f"psum_eq{g}")
                    nc.tensor.matmul(
                        out=pe[:, :],
                        lhsT=sel_sb[B : B + 2, 0:128],
                        rhs=rows[B : B + 2, c0 : c0 + CHUNK],
                        start=True,
                        stop=True,
                        tile_position=(B, 0),
                    )
                    psum_eqs.append(pe)
                pe4 = eqpool.tile([64, CHUNK], F32, tag="psum_eq4")
                nc.tensor.matmul(
                    out=pe4[:, :],
                    lhsT=sel_sb[0:1, 0:64],
                    rhs=rows2[0:1, c0 : c0 + CHUNK],
                    start=True,
                    stop=True,
                )

                # masked moving operands (bf16) and conv matmuls
                acc = accpool.tile([OC, CHUNK], F32, tag="acc")
                for g, (ta, tb, delta) in enumerate(GROUPS):
                    kh, kw = ta
                    src = inp3 if delta == WP else inp2
                    kpart = 128 if tb is not None else 64
                    in0 = src[0:kpart].rearrange("c (h w) -> c h w", h=HP)
                    in0 = in0[:, h0 + kh : h0 + kh + ROWS_PER_CHUNK, kw : kw + W]
                    peq = psum_eqs[g] if tb is not None else pe4
                    in1 = peq[0:kpart].rearrange("c (r w) -> c r w", r=ROWS_PER_CHUNK)
                    m = wpool.tile([kpart, CHUNK], BF16, tag=f"masked{g}")
                    mv = m[:, :].rearrange("c (r w) -> c r w", r=ROWS_PER_CHUNK)
                    nc.vector.tensor_tensor(
                        out=mv, in0=in0, in1=in1, op=AluOpType.mult
                    )
                    nc.tensor.matmul(
                        out=acc[:, :],
                        lhsT=w_sb[0:kpart, g * OC : (g + 1) * OC],
                        rhs=m[:, :],
                        start=(g == 0),
                        stop=(g == len(GROUPS) - 1),
                    )

                # bias + copy out
                ob = wpool.tile([OC, CHUNK], F32, tag="out_sb")
                nc.scalar.activation(
                    out=ob[:, :],
                    in_=acc[:, :],
                    func=ActivationFunctionType.Identity,
                    bias=bias_sb[:, :],
                    scale=1.0,
                )
                nc.sync.dma_start(out=out_d[:, c0 : c0 + CHUNK], in_=ob[:, :])

    nc.compile()
    return nc


_NC_CACHE = {}


def get_nc(repeat=1):
    if repeat not in _NC_CACHE:
        nc = bacc.Bacc("TRN2", target_bir_lowering=False)
        _NC_CACHE[repeat] = _build(nc, repeat=repeat)
    return _NC_CACHE[repeat]


def make_wpack(conv):
    """[128, 5*OC] f32: group g cols, rows 0:64 = tap_a weights (w[o,c,a]
    transposed to [c, o]), rows 64:128 = tap_b weights (zero for single)."""
    wpack = np.zeros((128, 5 * OC), dtype=np.float32)
    for g, (ta, tb, _d) in enumerate(GROUPS):
        wpack[0:C, g * OC : (g + 1) * OC] = conv[:, :, ta[0], ta[1]].T
        if tb is not None:
            wpack[64:128, g * OC : (g + 1) * OC] = conv[:, :, tb[0], tb[1]].T
    return wpack


def kernel(inp, mask, conv, bias):
    from concourse.bass_utils import run_bass_kernel_spmd

    inp = np.ascontiguousarray(inp, dtype=np.float32)
    mask = np.ascontiguousarray(mask, dtype=np.float32)
    conv = np.ascontiguousarray(conv, dtype=np.float32)
    bias = np.ascontiguousarray(bias, dtype=np.float32)
    B = inp.shape[0]

    nc = get_nc()
    wpack = make_wpack(conv)
    in_maps = [
        {
            "inp": inp[b].reshape(C, P),
            "mask": mask[b],
            "wpack": wpack,
            "bias": bias.reshape(OC, 1),
        }
        for b in range(B)
    ]
    res = run_bass_kernel_spmd(nc, in_maps, core_ids=list(range(B)))
    out = np.stack([res.results[b]["out"].reshape(OC, H, W) for b in range(B)])
    mask_out = mask.reshape(B, 1, H, W).copy()
    return out, mask_out


if __name__ == "__main__":
    # smoke build
    get_nc()
    print("build ok")
